# revision 1
# baseline (speedup 1.0000x reference)
"""Trainium2 Bass kernel for nn_Balancer_10660108829428.

Computes (total, fg_loss, bg_loss) for a fg/bg-weighted loss balancer:
  fg_mask[b,h,w] = any of 48 boxes covers pixel (h,w)
  fg_loss = 13 * sum(loss * fg) / (B*H*W)
  bg_loss = sum(loss * ~fg) / (B*H*W)
  total   = fg_loss + bg_loss

Strategy: data-parallel over B across 8 NeuronCores (8 batches each).
Per core each batch's mask is rasterized as a rank-48 matmul
(row_inT @ col_in) on the TensorEngine in bf16; a fused DVE/Pool
scalar_tensor_tensor computes (counts > 0) * loss with a free
per-partition row-sum accumulator; ScalarE accumulates the all-pixel
sums for pairs 0-2 via activation(Copy, accum_out=...).

The kernel is DMA-bound (15 MB of loss per core at ~360 GB/s), so the
endgame is scheduled to drain near DMA line rate:
 - masks are fp16 (iota integers <= 2048 exact) with the wide column
   masks on DVE and rows on Pool, so mask building never gates pair 2+;
 - pair 3's fg masks for hc0-q1 and the tail are precomputed as
   sign(counts) fp16 tiles via ScalarE Sign passes tucked into earlier
   chunk slots, decoupling the endgame from the PSUM count rotation;
 - pair 3's all-sums run on the TensorEngine as ones^T @ loss f32r
   matmuls (1 cycle/col at n=256) accumulating into [1,*] PSUM
   prefixes; hc0-q1's and tail-b6A's fg terms are Pool products
   (mask x loss -> f32r) reduced the same way; ScalarE folds each
   prefix into the accumulator between its producer groups;
 - the last h-chunk is DMA'd as eight column pieces whose fg-stts run
   on DVE at line rate, with the final piece's all-sum as a DVE
   background pass (host adds fg+bg), so only small ops trail the
   final DMA byte.
The final reduction of the [128,48] accumulator tile is done on the
host (it is pure gather arithmetic over named column groups).

Box membership avoids floor/ceil entirely: for integer h,
  h >= floor(v1)  <=>  h > v1 - 1      and      h < ceil(v2)  <=>  h < v2.
"""

import numpy as np

import concourse.bacc as bacc
import concourse.mybir as mybir
import concourse.tile as tile
from concourse.bass_utils import run_bass_kernel_spmd

B, H, W, N = 64, 376, 1248, 48
N_CORES = 8
BPC = B // N_CORES          # batches per core
PAIRS = BPC // 2            # batch pairs per core (masks built 2 batches at a time)
FG_WEIGHT = 13.0
H_CHUNKS = [(0, 128), (128, 128), (256, H - 256)]  # (h0, hsz)
F32 = mybir.dt.float32
F32R = mybir.dt.float32r
BF16 = mybir.dt.bfloat16
FP16 = mybir.dt.float16

# accumulator column layout (host sums these)
#  0..27  : fg partials   (27 = tail b7C fg, also counted into s_all)
#  32..44 : all-pixel partials (Act salls, PSUM closes, direct sall, b7C bg)
OUTC = 46
FG_LO, FG_HI = 0, 30
ALL_LO, ALL_HI = 32, 46
# fg columns of tail pieces whose all-sum was computed as fg+bg: the host
# adds these into s_all alongside the bg columns.
BG_PAIRED_FG_COLS = (28, 29)

_NC_CACHE = None


def _build_nc():
    # Bacc (not bass.Bass): its finalize() runs the TRN2 wait-legalization
    # passes (move_matmul_waits_to_ldweights / generate_event_semaphores) —
    # the ISA allows only one semaphore wait per instruction.
    nc = bacc.Bacc("TRN2")
    loss_d = nc.dram_tensor("loss", [BPC, H, W], F32, kind="ExternalInput")
    # boxes arrive host-transposed to the on-chip (q*64+n, 4*p+c) layout so
    # the DMA descriptors are contiguous 64 B runs instead of 16 B scatters
    boxes_d = nc.dram_tensor("boxes", [2, N, 4 * PAIRS], F32, kind="ExternalInput")
    out_d = nc.dram_tensor("out", [128, OUTC], F32, kind="ExternalOutput")

    AX = mybir.AxisListType
    OP = mybir.AluOpType
    AF = mybir.ActivationFunctionType

    with tile.TileContext(nc) as tc:
        with (
            tc.tile_pool(name="singles", bufs=1) as singles,
            tc.tile_pool(name="masks", bufs=4) as masks,
            tc.tile_pool(name="ltiles", bufs=7) as ltiles,
            tc.tile_pool(name="scratch", bufs=2) as scratch,
            tc.tile_pool(name="sfpool", bufs=4) as sfpool,
            tc.tile_pool(name="cpsum", bufs=2, space="PSUM") as cpsum,
            tc.tile_pool(name="spsum", bufs=1, space="PSUM") as spsum,
        ):
            # --- constants ---
            iota_i = singles.tile([128, W], mybir.dt.int32)
            nc.gpsimd.iota(iota_i, pattern=[[1, W]], base=0, channel_multiplier=0)
            # fp16 iota: integers <= 2048 are exact, and 2-byte operands let
            # the DVE mask compares run in 4x mode
            iota_f = singles.tile([128, W], FP16)
            nc.vector.tensor_copy(iota_f, iota_i)
            ones = singles.tile([128, 1], F32)
            nc.vector.memset(ones, 1.0)
            # f32r view for the PE all-sum matmuls: the BIR verifier requires
            # every fp32r-matmul input to be PRODUCED with f32r output dtype,
            # so ones gets a rounded copy and the pair-3 loss DMAs declare
            # their SBUF output APs as f32r (same bits; non-matmul readers
            # keep reading the tile as plain f32).
            ones_r = singles.tile([128, 1], F32R)
            nc.vector.tensor_copy(ones_r, ones)
            # accum slots are written (not accumulated) by accum_out for the
            # partitions each op covers; rows past hsz keep this zero fill.
            acc = singles.tile([128, OUTC], F32)
            nc.vector.memset(acc, 0.0)
            # PSUM prefix accumulators for the PE-side all-sums (pair 3):
            # ps_a collects hc0/hc1 and is closed mid-stream; ps_b collects
            # the tail pieces and gets a cheap [1,256] close at the end.
            # Each psum tile hosts SEQUENTIAL accumulation groups (a new
            # group's start=True matmul is ordered after the prior group's
            # Act close by the WAR dependency): ps_a = hc0/hc1 all-sums,
            # then tail-b6A's fg-reduce; ps_b = hc0-q1's fg-reduce, then
            # the tail all-sums.
            ps_a = spsum.tile([1, 512], F32, tag="psa")
            ps_b = spsum.tile([1, 512], F32, tag="psb")

            # batch-in-pair q lives at partition base 64*q (matmul requires
            # operand base partitions of 0/32/64); partitions 48..63 are
            # zeroed padding.
            NP = 64 + N  # 112 partitions spanned by the two batches

            # All boxes in two DMAs, already in the (q*64+n) partition layout
            # used by the mask builds: bx_all[q*64+n, 4*p+c] = boxes[2p+q, n, c].
            bx_all = singles.tile([128, 4 * PAIRS], F32)
            nc.vector.memset(bx_all, 0.0)
            for q in range(2):
                nc.sync.dma_start(
                    out=bx_all[64 * q : 64 * q + N, :],
                    in_=boxes_d[q],
                )
            # (u1-1, v1-1) per box-instance, all pairs in one op
            bm1_all = singles.tile([128, 2 * PAIRS], F32)
            nc.vector.tensor_scalar(
                bm1_all[:NP].rearrange("n (p c) -> n p c", p=PAIRS),
                bx_all[:NP].rearrange("n (p c) -> n p c", p=PAIRS)[:, :, 0:2],
                1.0,
                None,
                OP.subtract,
            )

            def build_masks(p, row_eng, col_eng):
                bx = bx_all[:, 4 * p : 4 * (p + 1)]
                bm1 = bm1_all[:, 2 * p : 2 * (p + 1)]
                # rows: (h > v1-1) & (h < v2)  as bf16
                rowa = masks.tile([128, H], FP16, tag="rowa")
                row_eng.tensor_scalar(
                    rowa[:NP], iota_f[:NP, :H], bm1[:NP, 1:2], None, OP.is_gt
                )
                rowb = masks.tile([128, H], FP16, tag="rowb")
                row_eng.tensor_scalar(
                    rowb[:NP], iota_f[:NP, :H], bx[:NP, 3:4], None, OP.is_lt
                )
                rowm = masks.tile([128, H], FP16, tag="rowm")
                row_eng.tensor_tensor(rowm[:NP], rowa[:NP], rowb[:NP], OP.mult)
                # cols: (w > u1-1) & (w < u2)  as bf16
                cola = masks.tile([128, W], FP16, tag="cola")
                col_eng.tensor_scalar(
                    cola[:NP], iota_f[:NP, :], bm1[:NP, 0:1], None, OP.is_gt
                )
                colb = masks.tile([128, W], FP16, tag="colb")
                col_eng.tensor_scalar(
                    colb[:NP], iota_f[:NP, :], bx[:NP, 2:3], None, OP.is_lt
                )
                colm = masks.tile([128, W], FP16, tag="colm")
                col_eng.tensor_tensor(colm[:NP], cola[:NP], colb[:NP], OP.mult)
                return rowm, colm

            # Pool builds masks serially at ~8.3us/pair (its multiply runs at
            # 0.42 efficiency), so it gets p1 and p2 plus p3's cheap rows, in
            # an order that has each pair ready before its chunks arrive; DVE
            # takes p0 and p3's wide column masks (~1.7us each, fp16 4x mode).
            # p3's masks finish ~17us in, so the tail box-counts can be
            # precomputed mid-stream without stalling anything.
            prebuilt = [None] * PAIRS
            prebuilt[0] = build_masks(0, nc.vector, nc.vector)
            prebuilt[1] = build_masks(1, nc.gpsimd, nc.gpsimd)
            prebuilt[3] = build_masks(3, nc.gpsimd, nc.vector)
            prebuilt[2] = build_masks(2, nc.gpsimd, nc.gpsimd)

            def emit_cnt(rowm, colm, q, h0, hsz):
                cnt = cpsum.tile([128, W], F32, tag="cnt")
                for w0 in range(0, W, 512):
                    wsz = min(512, W - w0)
                    nc.tensor.matmul(
                        cnt[:hsz, w0 : w0 + wsz],
                        lhsT=rowm[64 * q : 64 * q + N, h0 : h0 + hsz],
                        rhs=colm[64 * q : 64 * q + N, w0 : w0 + wsz],
                        start=True,
                        stop=True,
                    )
                return cnt

            def fg_stt(eng, lt, cnt, hsz, q, c0, csz, col, op0=OP.is_gt):
                # fused (counts > 0) * loss with free per-partition row sums
                sf = sfpool.tile([128, W], F32, tag="sf")
                eng.scalar_tensor_tensor(
                    sf[:hsz, c0 : c0 + csz],
                    cnt[:hsz, c0 : c0 + csz],
                    0.0,
                    lt[:hsz, q * W + c0 : q * W + c0 + csz],
                    op0=op0,
                    op1=OP.mult,
                    accum_out=acc[:hsz, col : col + 1],
                )

            def act_sall(lt, hsz, f0, fsz, col):
                sa = scratch.tile([128, 2 * W], F32, tag="sa")
                nc.scalar.activation(
                    out=sa[:hsz, :fsz],
                    in_=lt[:hsz, f0 : f0 + fsz],
                    func=AF.Copy,
                    accum_out=acc[:hsz, col : col + 1],
                )

            # PE all-sum: ones^T @ loss-piece, f32r (1 cyc/col for n>=256),
            # every piece accumulates into the [0:n] prefix of its psum.
            pe_first = {"psa": True, "psa2": True, "psb1": True, "psb2": True}

            def pe_acc(ps, key, rhs_ap, hsz, n, first=None, last=False):
                # ones^T @ rhs piece -> [1, n] prefix accumulation in PSUM.
                # rhs must be f32r-typed (f32r-bitcast DMA or Pool f32r out).
                if first is None:
                    first = pe_first[key]
                nc.tensor.matmul(
                    ps[0:1, 0:n],
                    lhsT=ones_r[:hsz, 0:1],
                    rhs=rhs_ap,
                    start=first,
                    stop=last,
                    skip_group_check=True,
                )
                pe_first[key] = False

            def pe_sall(ps, key, lt, hsz, f0, n, last=False):
                pe_acc(ps, key, lt[:hsz, f0 : f0 + n].bitcast(F32R), hsz, n,
                       last=last)

            def act_close(ps, n, col):
                # fold a PE psum prefix into one accumulator column
                cls = scratch.tile([128, 2 * W], F32, tag="sa")
                nc.scalar.activation(
                    out=cls[0:1, 0:n],
                    in_=ps[0:1, 0:n],
                    func=AF.Copy,
                    accum_out=acc[0:1, col : col + 1],
                )

            # ---------------- pairs 0-2: steady state -------------------
            # Pair 3's fg masks for hc0-q1 and the tail are precomputed into
            # SBUF as fp16 0/1 masks via Act Sign passes (counts >= 0, so
            # sign(cnt) is exactly the fg mask) that fit in Act's per-chunk
            # slack. Slots are chosen so each prep's PSUM cnt-buffer hold
            # fits between neighbouring chunks' counts, and pair 3's three
            # remaining in-PSUM counts all land before their stts need them.
            cnt_sb = {}
            prep_specs = {
                (1, 1): ("t6", 2, 0),
                (1, 2): ("t7", 2, 1),
                (2, 0): ("hc0q1", 0, 1),
            }
            for p in range(3):
                rowm, colm = prebuilt[p]
                for hc, (h0, hsz) in enumerate(H_CHUNKS):
                    lt = ltiles.tile([128, 2 * W], F32, tag="lt")
                    nc.sync.dma_start(
                        out=lt[:hsz].rearrange("h (b w) -> h b w", b=2),
                        in_=loss_d[2 * p : 2 * p + 2, h0 : h0 + hsz, :].rearrange(
                            "b h w -> h b w"
                        ),
                    )
                    act_sall(lt, hsz, 0, 2 * W, 32 + p * 3 + hc)
                    for q in range(2):
                        cnt = emit_cnt(rowm, colm, q, h0, hsz)
                        fg_stt(nc.vector, lt, cnt, hsz, q, 0, W, p * 6 + hc * 2 + q)
                    if (p, hc) in prep_specs:
                        key, hct, qt = prep_specs[(p, hc)]
                        rowm3, colm3 = prebuilt[3]
                        h0t, hszt = H_CHUNKS[hct]
                        cntp = emit_cnt(rowm3, colm3, qt, h0t, hszt)
                        csb = singles.tile([128, W], FP16, tag=f"cntsb{key}")
                        nc.scalar.activation(
                            out=csb[:hszt], in_=cntp[:hszt], func=AF.Sign
                        )
                        cnt_sb[key] = csb

            # ---------------- pair 3: balanced endgame ------------------
            rowm, colm = prebuilt[3]

            # hc0: one DMA; q0 fg on DVE (PSUM cnt); q1 fg as a Pool product
            # (Sign-mask x loss -> f32r) reduced on PE into ps_d and closed
            # early by Act; all-sums on PE into ps_a.
            h0, hsz = H_CHUNKS[0]
            lt0 = ltiles.tile([128, 2 * W], F32, tag="lt")
            nc.sync.dma_start(
                out=lt0[:hsz].rearrange("h (b w) -> h b w", b=2).bitcast(F32R),
                in_=loss_d[6:8, h0 : h0 + hsz, :]
                .rearrange("b h w -> h b w")
                .bitcast(F32R),
            )
            cnt_q0 = emit_cnt(rowm, colm, 0, h0, hsz)
            fg_stt(nc.vector, lt0, cnt_q0, hsz, 0, 0, W, 18)
            mlt0 = singles.tile([128, W], F32R, tag="mlt0")
            nc.gpsimd.tensor_tensor(
                mlt0[:hsz], cnt_sb["hc0q1"][:hsz], lt0[:hsz, W : 2 * W], OP.mult
            )
            h0h, hszh = H_CHUNKS[1]
            lt1 = ltiles.tile([128, 2 * W], F32, tag="lt")
            nc.sync.dma_start(
                out=lt1[:hszh, 0:W].bitcast(F32R),
                in_=loss_d[6, h0h : h0h + hszh, :].bitcast(F32R),
            )
            nc.sync.dma_start(
                out=lt1[:hszh, W : 2 * W].bitcast(F32R),
                in_=loss_d[7, h0h : h0h + hszh, :].bitcast(F32R),
            )
            # hc0's all-sum goes to Act (idle here), keeping PE free for
            # the counts, fg-reduces and hc1/tail all-sums that follow
            act_sall(lt0, hsz, 0, 2 * W, 45)
            # hc1's counts are emitted before the hc0-q1 fg-reduce block so
            # the PE queue never blocks them behind the Pool product
            cnt_b6 = emit_cnt(rowm, colm, 0, h0h, hszh)
            fg_stt(nc.vector, lt1, cnt_b6, hszh, 0, 0, W, 20)
            cnt_b7 = emit_cnt(rowm, colm, 1, h0h, hszh)
            fg_stt(nc.vector, lt1, cnt_b7, hszh, 1, 0, W, 21)
            for c0, n in ((0, 256), (256, 256), (512, 256), (768, 256),
                          (1024, 224)):
                pe_acc(ps_b, "psb1", mlt0[:hsz, c0 : c0 + n], hsz, n,
                       last=c0 == 1024)
            act_close(ps_b, 256, 19)
            h0, hsz = h0h, hszh
            for q in range(2):
                for c0, n in ((0, 256), (256, 256), (512, 256), (768, 256),
                              (1024, 224)):
                    pe_sall(ps_a, "psa", lt1, hsz, q * W + c0, n,
                            last=q == 1 and c0 == 1024)
            act_close(ps_a, 256, 41)

            # hc2 (tail, 120 rows): eight column pieces, b6A's fg via
            # Pool-product + PE (closed by Act mid-tail), the rest on DVE at
            # line rate; all-sums on PE into ps_b (closed once its last
            # writer lands) except b7C (Act direct) and b7D (DVE bg pass).
            h0, hsz = H_CHUNKS[2]
            lt2 = ltiles.tile([128, 2 * W], F32, tag="lt")
            fgm6, fgm7 = cnt_sb["t6"], cnt_sb["t7"]
            fgms = (fgm6, fgm7)
            # (q, c0, csz, fg_kind, fg_col, sall_kind, sall_col)
            pieces = [
                (0, 0, 512, "pool", 22, "pe", None),
                (1, 0, 512, "dve", 23, "pe", None),
                (0, 512, 288, "dve", 24, "pe", None),
                (0, 800, 224, "dve", 25, "pe", None),
                (1, 512, 288, "dve", 27, "pe", None),
                (0, 1024, 224, "dve", 26, "pe_last", None),
                (1, 800, 224, "dve", 28, "dvebg", 43),
                (1, 1024, 224, "dve", 29, "dvebg", 44),
            ]
            for q, c0, csz, fgk, fcol, sk, scol in pieces:
                nc.sync.dma_start(
                    out=lt2[:hsz, q * W + c0 : q * W + c0 + csz].bitcast(F32R),
                    in_=loss_d[6 + q, h0 : h0 + hsz, c0 : c0 + csz].bitcast(F32R),
                )
            mlt1 = singles.tile([128, 512], F32R, tag="mlt1")
            mlt2 = singles.tile([128, 288], F32R, tag="mlt2")
            for q, c0, csz, fgk, fcol, sk, scol in pieces:
                if fgk in ("pool", "pool2"):
                    mlt = mlt1 if fgk == "pool" else mlt2
                    nc.gpsimd.tensor_tensor(
                        mlt[:hsz, 0:csz],
                        fgms[q][:hsz, c0 : c0 + csz],
                        lt2[:hsz, q * W + c0 : q * W + c0 + csz],
                        OP.mult,
                    )
                else:
                    fg_stt(nc.vector, lt2, fgms[q], hsz, q, c0, csz, fcol)
                if sk in ("pe", "pe_last"):
                    for cc in range(c0, c0 + csz, 256):
                        nn_ = min(256, c0 + csz - cc)
                        pe_sall(ps_b, "psb2", lt2, hsz, q * W + cc, nn_,
                                last=sk == "pe_last" and cc + 256 >= c0 + csz)
                elif sk == "act":
                    act_sall(lt2, hsz, q * W + c0, csz, scol)
                elif sk == "dvebg":
                    # all-sum(piece) = fg + bg; host adds the fg column too
                    fg_stt(nc.vector, lt2, fgms[q], hsz, q, c0, csz, scol,
                           op0=OP.is_le)
                if fgk == "pool" or (q, c0) == (0, 800):
                    # b6A's PE fg-reduce: emitted after b6C's sall so the PE
                    # queue never stalls waiting for the Pool product
                    if (q, c0) == (0, 800):
                        # fg-reduces of both Pool products (emitted here so
                        # the PE queue never stalls waiting for Pool)
                        for cc in (0, 256):
                            pe_acc(ps_a, "psa2", mlt1[:hsz, cc : cc + 256],
                                   hsz, 256, first=cc == 0, last=cc == 256)
                        act_close(ps_a, 256, 22)
            act_close(ps_b, 256, 42)

            nc.sync.dma_start(out=out_d[:, :], in_=acc)

    nc.finalize()
    return nc


def get_nc():
    global _NC_CACHE
    if _NC_CACHE is None:
        _NC_CACHE = _build_nc()
    return _NC_CACHE


def run_cores(loss, gt_boxes2d, trace=False, **kw):
    loss = np.ascontiguousarray(loss, dtype=np.float32)
    boxes = np.ascontiguousarray(gt_boxes2d, dtype=np.float32)
    in_maps = []
    for c in range(N_CORES):
        bc = boxes[c * BPC : (c + 1) * BPC]  # [BPC, N, 4]
        # host-side permutation to the kernel's (q, n, p, c) layout
        bt = bc.reshape(PAIRS, 2, N, 4).transpose(1, 2, 0, 3)
        in_maps.append(
            {
                "loss": np.ascontiguousarray(loss[c * BPC : (c + 1) * BPC]),
                "boxes": np.ascontiguousarray(bt.reshape(2, N, 4 * PAIRS)),
            }
        )
    return run_bass_kernel_spmd(
        get_nc(), in_maps, core_ids=list(range(N_CORES)), trace=trace, **kw
    )


def kernel(loss, gt_boxes2d):
    res = run_cores(loss, gt_boxes2d)
    s_fg = 0.0
    s_all = 0.0
    for r in res.results:
        o = np.asarray(r["out"], dtype=np.float64)
        s_fg += float(o[:, FG_LO:FG_HI].sum())
        s_all += float(o[:, ALL_LO:ALL_HI].sum())
        for c in BG_PAIRED_FG_COLS:
            s_all += float(o[:, c].sum())
    n_pix = float(B * H * W)
    fg_loss = FG_WEIGHT * s_fg / n_pix
    bg_loss = (s_all - s_fg) / n_pix
    total = fg_loss + bg_loss
    return (
        np.array(total, dtype=np.float32),
        np.array(fg_loss, dtype=np.float32),
        np.array(bg_loss, dtype=np.float32),
    )

